# revision 1
# baseline (speedup 1.0000x reference)
"""Bass/Tile TRN2 kernel for nn_LocalTransformerBlock.

Sharding: pure data-parallel — batch B=8, one batch element per NeuronCore.
Per-core: full transformer block on (4096, 512) in 32 row-tiles of 128 tokens
(window size == tile size). Matmuls run in float32r (full PE speed at free
dim >= 256); elementwise in fp32. LN gains are folded into the weight
matrices host-side; rope tables carry q/k scales and the 8.0 QK scale.
"""
import numpy as np
from contextlib import ExitStack

import concourse.bass as bass
import concourse.bacc as bacc
import concourse.tile as tile
from concourse import masks as cmasks
from concourse import mybir
from concourse.bass_utils import run_bass_kernel_spmd

DIM = 512
HEADS = 8
DHEAD = 64
WIN = 128
NTOK = 4096
NT = NTOK // WIN          # 32 row tiles
B = 8
LN_EPS = 1e-5
QK_SCALE = 8.0
NEG = -30000.0

F32 = mybir.dt.float32
F32R = mybir.dt.float32r
BF16 = mybir.dt.bfloat16
F16 = mybir.dt.float16
AF = mybir.ActivationFunctionType


def _bc(ap, dims):
    """Rebuild an AP with explicit [step, count] dims (for broadcasts)."""
    return bass.AP(tensor=ap.tensor, offset=ap.offset, ap=dims)


def build_program(has_qkv_bias, has_ff_bias, has_out_bias):
    nc = bacc.Bacc()

    x_d = nc.declare_dram_parameter("x", [NTOK, DIM], F32, isOutput=False)
    wqkvT_d = nc.declare_dram_parameter("wqkvT", [DIM, 3 * DIM], F32R, isOutput=False)
    woutT_d = nc.declare_dram_parameter("woutT", [DIM, DIM], F32R, isOutput=False)
    wff1T_d = nc.declare_dram_parameter("wff1T", [DIM, 4 * DIM], F32R, isOutput=False)
    wff2T_d = nc.declare_dram_parameter("wff2T", [4 * DIM, DIM], BF16, isOutput=False)
    rope_d = nc.declare_dram_parameter("rope", [NTOK, 4 * DHEAD], F32, isOutput=False)
    masks_d = nc.declare_dram_parameter("masks", [2, WIN, 2 * WIN], F32, isOutput=False)
    bias_d = None
    if has_qkv_bias or has_ff_bias or has_out_bias:
        bias_d = nc.declare_dram_parameter("biases", [3 * DIM + DIM + DIM], F32,
                                           isOutput=False)
    out_d = nc.declare_dram_parameter("out", [NTOK, DIM], F32, isOutput=True)

    with ExitStack() as ctx:
        tc = ctx.enter_context(tile.TileContext(nc))
        consts = ctx.enter_context(tc.tile_pool(name="consts", bufs=1))
        io = ctx.enter_context(tc.tile_pool(name="io", bufs=2))
        work = ctx.enter_context(tc.tile_pool(name="work", bufs=2))
        xpool = ctx.enter_context(tc.tile_pool(name="xpool", bufs=3))
        w512 = ctx.enter_context(tc.tile_pool(name="w512", bufs=2))
        slab = ctx.enter_context(tc.tile_pool(name="slab", bufs=8))
        gpool = ctx.enter_context(tc.tile_pool(name="gpool", bufs=2))
        small = ctx.enter_context(tc.tile_pool(name="small", bufs=4))
        psU = ctx.enter_context(tc.tile_pool(name="psU", bufs=8, space="PSUM"))

        # ---- resident constants ----
        wq_sb = consts.tile([128, 4, 3 * DIM], F32R)
        wo_sb = consts.tile([64, 8, DIM], F32R)
        wf1_sb = consts.tile([128, 4, 4 * DIM], F32R)
        wf2_sb = consts.tile([128, 16, DIM], BF16)
        for k in range(4):
            nc.sync.dma_start(out=wq_sb[:, k, :], in_=wqkvT_d[k * 128:(k + 1) * 128, :])
            nc.sync.dma_start(out=wf1_sb[:, k, :], in_=wff1T_d[k * 128:(k + 1) * 128, :])
        for k in range(16):
            nc.sync.dma_start(out=wf2_sb[:, k, :], in_=wff2T_d[k * 128:(k + 1) * 128, :])
        for hd in range(8):
            nc.sync.dma_start(out=wo_sb[:, hd, :], in_=woutT_d[hd * 64:(hd + 1) * 64, :])
        eye_sb = consts.tile([128, 128], F32)
        cmasks.make_identity(nc, eye_sb[:, :])
        eye_bf = consts.tile([128, 128], BF16)
        cmasks.make_identity(nc, eye_bf[:, :])
        eye_f16 = consts.tile([128, 128], F16)
        cmasks.make_identity(nc, eye_f16[:, :])
        mask_sb = consts.tile([128, 2, 2 * WIN], F32)
        nc.sync.dma_start(out=mask_sb, in_=masks_d.rearrange("m p j -> p m j"))
        bias_sb = None
        if bias_d is not None:
            bias_sb = consts.tile([128, 3 * DIM + 2 * DIM], F32)
            nc.sync.dma_start(out=bias_sb,
                              in_=_bc(bias_d[:], [[0, 128], [1, 3 * DIM + 2 * DIM]]))

        # k/v rings: slot t%2 holds tile t's keys in [:, :, WIN:] and tile
        # t+1's look-back copy lands in slot (t+1)%2 at [:, :, :WIN].
        kwin = [consts.tile([64, HEADS, 2 * WIN], F32R, name=f"kwin{i}") for i in range(2)]
        vwin = [consts.tile([128, HEADS * DHEAD], F16, name=f"vwin{i}") for i in range(2)]
        eps_ln = consts.tile([128, 1], F32, name="eps_ln")
        nc.vector.memset(eps_ln, LN_EPS)
        eps_sq = consts.tile([128, 1], F32, name="eps_sq")
        nc.vector.memset(eps_sq, 1e-24)

        def layernorm(src, tag):
            st = small.tile([128, nc.vector.BN_STATS_DIM], F32, name=f"st_{tag}")
            nc.vector.bn_stats(st, src)
            mv = small.tile([128, nc.vector.BN_AGGR_DIM], F32, name=f"mv_{tag}")
            nc.vector.bn_aggr(mv, st)
            sd = small.tile([128, 1], F32, name=f"sd_{tag}")
            nc.scalar.activation(out=sd, in_=mv[:, 1:2], func=AF.Sqrt, bias=eps_ln[:, 0:1])
            rstd = small.tile([128, 1], F32, name=f"rstd_{tag}")
            nc.vector.reciprocal(rstd, sd)
            h = w512.tile([128, DIM], F32, name="h_x", tag="h_x")
            nc.vector.tensor_scalar(out=h, in0=src, scalar1=mv[:, 0:1],
                                    scalar2=rstd, op0=mybir.AluOpType.subtract,
                                    op1=mybir.AluOpType.mult)
            return h

        def prep_qk(src512, rope_t, roff, dst_tag):
            """l2norm per head + scale/rope (baked into rope tables)."""
            s3 = src512.rearrange("p (h d) -> p h d", h=HEADS)
            sq = w512.tile([128, DIM], F32, name="sq")
            sq3 = sq.rearrange("p (h d) -> p h d", h=HEADS)
            nc.vector.tensor_mul(sq3, s3, s3)
            ss = small.tile([128, HEADS], F32, name="ss")
            nc.vector.tensor_reduce(out=ss, in_=sq3, axis=mybir.AxisListType.X,
                                    op=mybir.AluOpType.add)
            nc.scalar.activation(out=ss, in_=ss, func=AF.Sqrt, bias=eps_sq[:, 0:1])
            rn = small.tile([128, HEADS], F32, name="rn")
            nc.vector.reciprocal(rn, ss)
            rnB = _bc(rn[:, :], rn.ap + [[0, DHEAD]])
            qn = w512.tile([128, DIM], F32, name="qn")
            qn3 = qn.rearrange("p (h d) -> p h d", h=HEADS)
            nc.vector.tensor_mul(qn3, s3, rnB)
            cos = rope_t[:, roff:roff + DHEAD]
            sin = rope_t[:, roff + DHEAD:roff + 2 * DHEAD]
            cosB = _bc(cos, [cos.ap[0], [0, HEADS], cos.ap[1]])
            sinLoB = _bc(sin[:, 0:32], [sin.ap[0], [0, HEADS], [1, 32]])
            sinHiB = _bc(sin[:, 32:64], [sin.ap[0], [0, HEADS], [1, 32]])
            qr = w512.tile([128, DIM], F32, name=dst_tag)
            qr3 = qr.rearrange("p (h d) -> p h d", h=HEADS)
            nc.vector.tensor_mul(qr3, qn3, cosB)
            nc.gpsimd.tensor_mul(sq3[:, :, 0:32], qn3[:, :, 32:64], sinLoB)
            nc.gpsimd.tensor_mul(sq3[:, :, 32:64], qn3[:, :, 0:32], sinHiB)
            nc.vector.tensor_add(qr3, qr3, sq3)
            return qr

        x2s = {}

        def stage_a(t):
            cur, prv = t % 2, (t + 1) % 2

            x_t = io.tile([128, DIM], F32, name="x_t")
            nc.sync.dma_start(out=x_t, in_=x_d[t * 128:(t + 1) * 128, :])
            rope_t = io.tile([128, 4 * DHEAD], F32, name="rope_t")
            nc.sync.dma_start(out=rope_t, in_=rope_d[t * 128:(t + 1) * 128, :])

            # ---- LN1 + QKV ----
            h = layernorm(x_t, "ln1")
            hT = work.tile([128, 4, 128], F32R, name="hT")
            for k in range(4):
                pt = psU.tile([128, 512], F32, name="pu", tag="pu")[:, :128]
                nc.tensor.transpose((pt), (h[:, k * 128:(k + 1) * 128]), (eye_sb))
                nc.scalar.copy(out=hT[:, k, :], in_=pt)
            qk_sb = gpool.tile([128, 2, DIM], F32, name="qk_sb")
            for c in range(3):
                pm = psU.tile([128, 512], F32, name="pu", tag="pu")
                for k in range(4):
                    nc.tensor.matmul(pm, (hT[:, k, :]),
                                     (wq_sb[:, k, c * 512:(c + 1) * 512]),
                                     start=(k == 0), stop=(k == 3))
                if has_qkv_bias:
                    nc.vector.tensor_add(pm, pm, bias_sb[:, c * 512:(c + 1) * 512])
                if c < 2:
                    nc.scalar.copy(out=qk_sb[:, c, :], in_=pm)
                else:
                    nc.scalar.copy(out=vwin[cur], in_=pm)

            # ---- q/k prep ----
            qr = prep_qk(qk_sb[:, 0, :], rope_t, 0, "qr")
            kr = prep_qk(qk_sb[:, 1, :], rope_t, 2 * DHEAD, "kr")

            # ---- per-head transposes of q', k' ----
            qT = work.tile([64, HEADS, 128], F32R, name="qT")
            for hd in range(HEADS):
                pt = psU.tile([128, 512], F32, name="pu", tag="pu")[:, :128]
                nc.tensor.transpose((pt[:64, :]), (qr[:, hd * 64:(hd + 1) * 64]),
                                    (eye_sb))
                nc.scalar.copy(out=qT[:, hd, :], in_=pt[:64, :])
                pt2 = psU.tile([128, 512], F32, name="pu", tag="pu")[:, :128]
                nc.tensor.transpose((pt2[:64, :]), (kr[:, hd * 64:(hd + 1) * 64]),
                                    (eye_sb))
                nc.scalar.copy(out=kwin[cur][:, hd, WIN:], in_=pt2[:64, :])
                nc.scalar.copy(out=kwin[prv][:, hd, :WIN], in_=pt2[:64, :])

            # ---- attention: all S matmuls first, then softmax/AV ----
            PTsb = work.tile([64, HEADS, 128], F32R, name="PTsb")
            As = []
            for hd in range(HEADS):
                ps = psU.tile([128, 512], F32, name="pu", tag="pu")[:, :2 * WIN]
                if t == 0:
                    nc.vector.memset(ps[:, 0:WIN], 0.0)
                    nc.tensor.matmul(ps[:, WIN:], (qT[:, hd, :]),
                                     (kwin[cur][:, hd, WIN:]),
                                     start=True, stop=True)
                else:
                    nc.tensor.matmul(ps, (qT[:, hd, :]), (kwin[cur][:, hd, :]),
                                     start=True, stop=True)
                nc.vector.tensor_add(ps, ps, mask_sb[:, 0 if t == 0 else 1, :])
                A = work.tile([128, 2 * WIN], F16, name=f"A{hd % 4}",
                              tag=f"A{hd % 4}")
                rs = small.tile([128, 1], F32, name="rs")
                nc.scalar.activation(out=A, in_=ps, func=AF.Exp, accum_out=rs)
                ri = small.tile([128, 1], F32, name="ri")
                nc.vector.reciprocal(ri, rs)
                nc.vector.tensor_scalar_mul(A, A, ri)
                As.append(A)
            for hd in range(HEADS):
                A = As[hd]
                AT = work.tile([128, 2 * WIN], F16, name="AT")
                for b2 in range(2):
                    pt = psU.tile([128, 512], F32, name="pu", tag="pu")
                    ptf = pt[:, :64].bitcast(F16)
                    nc.tensor.transpose(ptf, (A[:, b2 * 128:(b2 + 1) * 128]),
                                        (eye_f16))
                    nc.scalar.copy(out=AT[:, b2 * 128:(b2 + 1) * 128], in_=ptf)
                pp = psU.tile([128, 512], F32, name="pu", tag="pu")[:64, :128]
                vsl = slice(hd * DHEAD, (hd + 1) * DHEAD)
                if t == 0:
                    nc.tensor.matmul(pp, (vwin[cur][:, vsl]), (AT[:, WIN:]),
                                     start=True, stop=True)
                else:
                    nc.tensor.matmul(pp, (vwin[prv][:, vsl]), (AT[:, 0:WIN]),
                                     start=True, stop=False)
                    nc.tensor.matmul(pp, (vwin[cur][:, vsl]), (AT[:, WIN:]),
                                     start=False, stop=True)
                nc.scalar.copy(out=PTsb[:, hd, :], in_=pp)

            # ---- output projection + residual ----
            py = psU.tile([128, 512], F32, name="pu", tag="pu")
            for hd in range(HEADS):
                nc.tensor.matmul(py, (PTsb[:, hd, :]), (wo_sb[:, hd, :]),
                                 start=(hd == 0), stop=(hd == 7))
            if has_out_bias:
                nc.vector.tensor_add(py, py, bias_sb[:, 3 * DIM:4 * DIM])
            x2 = xpool.tile([128, DIM], F32, name="x2", tag="x2")
            nc.vector.tensor_add(x2, x_t, py)
            x2s[t] = x2

        def stage_b(t):
            x2 = x2s.pop(t)

            # ---- FFN ----
            h2 = layernorm(x2, "ln2")
            h2T = work.tile([128, 4, 128], F32R, name="h2T")
            for k in range(4):
                pt = psU.tile([128, 512], F32, name="pu", tag="pu")[:, :128]
                nc.tensor.transpose((pt), (h2[:, k * 128:(k + 1) * 128]), (eye_sb))
                nc.scalar.copy(out=h2T[:, k, :], in_=pt)
            g = gpool.tile([128, 4 * DIM], BF16, name="g")
            for c in range(4):
                pf = psU.tile([128, 512], F32, name="pu", tag="pu")
                for k in range(4):
                    nc.tensor.matmul(pf, (h2T[:, k, :]),
                                     (wf1_sb[:, k, c * 512:(c + 1) * 512]),
                                     start=(k == 0), stop=(k == 3))
                if has_ff_bias:
                    nc.vector.tensor_add(pf, pf, bias_sb[:, 4 * DIM + c * 512:
                                                         4 * DIM + (c + 1) * 512])
                nc.scalar.activation(out=g[:, c * 512:(c + 1) * 512], in_=pf,
                                     func=AF.Gelu)
            py2 = psU.tile([128, 512], F32, name="pu", tag="pu")
            for kb in range(4):
                gss = []
                for k4 in range(4):
                    k = kb * 4 + k4
                    pt = psU.tile([128, 512], F32, name="pu", tag="pu")
                    ptb = pt[:, :64].bitcast(BF16)
                    nc.tensor.transpose(ptb, (g[:, k * 128:(k + 1) * 128]),
                                        (eye_bf))
                    gs = slab.tile([128, 128], BF16, name="gs")
                    nc.scalar.copy(out=gs, in_=ptb)
                    gss.append(gs)
                for k4 in range(4):
                    k = kb * 4 + k4
                    nc.tensor.matmul(py2, (gss[k4]), (wf2_sb[:, k, :]),
                                     start=(k == 0), stop=(k == 15))
            out_t = work.tile([128, DIM], F32, name="out_t", tag="out_t")
            nc.vector.tensor_add(out_t, x2, py2)
            nc.sync.dma_start(out=out_d[t * 128:(t + 1) * 128, :], in_=out_t)

        stage_a(0)
        stage_a(1)
        for t in range(2, NT):
            stage_a(t)
            stage_b(t - 2)
        stage_b(NT - 2)
        stage_b(NT - 1)

    nc.compile()
    return nc


_CACHE = {}


def prepare(x, w_qkv, q_scale, k_scale, w_out, b_out, ln1_g, ln1_b,
            ff_ln_g, ff_ln_b, w_ff1, w_ff2):
    x = np.asarray(x, np.float32)

    # ---- host-side folding ----
    ln1_g = np.asarray(ln1_g, np.float32)
    ln1_b = np.asarray(ln1_b, np.float32)
    ff_ln_g = np.asarray(ff_ln_g, np.float32)
    ff_ln_b = np.asarray(ff_ln_b, np.float32)
    w_qkv = np.asarray(w_qkv, np.float32)
    w_ff1 = np.asarray(w_ff1, np.float32)
    wqkvT = np.ascontiguousarray((w_qkv * ln1_g[None, :]).T)          # (512,1536)
    woutT = np.ascontiguousarray(np.asarray(w_out, np.float32).T)     # (512,512)
    wff1T = np.ascontiguousarray((w_ff1 * ff_ln_g[None, :]).T)        # (512,2048)
    from concourse import mybir as _mybir
    _bf = _mybir.dt.np(_mybir.dt.bfloat16)
    wff2T = np.ascontiguousarray(np.asarray(w_ff2, np.float32).T).astype(_bf)
    bias_qkv = w_qkv @ ln1_b                                          # (1536,)
    bias_ff = w_ff1 @ ff_ln_b                                         # (2048,)
    b_out = np.asarray(b_out, np.float32)
    has_qkv_bias = bool(np.any(bias_qkv))
    has_ff_bias = bool(np.any(bias_ff))
    has_out_bias = bool(np.any(b_out))
    biases = np.concatenate([bias_qkv, b_out, bias_ff]).astype(np.float32)

    # rope tables with l2norm-scale and QK_SCALE baked in
    pos = np.arange(NTOK, dtype=np.float32)
    inv_freq = 1.0 / (10000.0 ** (np.arange(0, DHEAD, 2, dtype=np.float32) / DHEAD))
    freqs = pos[:, None] * inv_freq
    emb = np.concatenate([freqs, freqs], axis=-1)                     # (NTOK, 64)
    cos, sin = np.cos(emb), np.sin(emb)
    qs = np.asarray(q_scale, np.float32)
    ks = np.asarray(k_scale, np.float32)
    rp = np.concatenate([qs[32:], qs[:32]])                           # rotperm
    kp = np.concatenate([ks[32:], ks[:32]])
    sgn = np.concatenate([-np.ones(32, np.float32), np.ones(32, np.float32)])
    qcos = cos * qs[None, :] * QK_SCALE
    qsin = sin * rp[None, :] * sgn[None, :] * QK_SCALE
    kcos = cos * ks[None, :]
    ksin = sin * kp[None, :] * sgn[None, :]
    rope = np.concatenate([qcos, qsin, kcos, ksin], axis=1).astype(np.float32)

    # additive masks: [0] first window (no look-back), [1] the rest
    i_idx = np.arange(WIN)[:, None]
    j_idx = np.arange(WIN)[None, :]
    causal = np.where(i_idx >= j_idx, 0.0, NEG).astype(np.float32)
    m_first = np.concatenate([np.full((WIN, WIN), NEG, np.float32), causal], axis=1)
    m_rest = np.concatenate([np.zeros((WIN, WIN), np.float32), causal], axis=1)
    masks = np.stack([m_first, m_rest])

    key = (has_qkv_bias, has_ff_bias, has_out_bias)
    if key not in _CACHE:
        _CACHE[key] = build_program(*key)
    nc = _CACHE[key]

    shared = dict(wqkvT=wqkvT, woutT=woutT, wff1T=wff1T, wff2T=wff2T,
                  rope=rope, masks=masks)
    if key != (False, False, False):
        shared["biases"] = biases
    in_maps = [dict(x=np.ascontiguousarray(x[i]), **shared) for i in range(B)]
    return nc, in_maps


def kernel(x, w_qkv, q_scale, k_scale, w_out, b_out, ln1_g, ln1_b,
           ff_ln_g, ff_ln_b, w_ff1, w_ff2, **run_kwargs):
    nc, in_maps = prepare(x, w_qkv, q_scale, k_scale, w_out, b_out, ln1_g,
                          ln1_b, ff_ln_g, ff_ln_b, w_ff1, w_ff2)
    res = run_bass_kernel_spmd(nc, in_maps, list(range(B)), **run_kwargs)
    out = np.stack([res.results[i]["out"] for i in range(B)]).astype(np.float32)
    if run_kwargs:
        return out, res
    return out



# revision 16
# speedup vs baseline: 1.0929x; 1.0929x over previous
"""Bass/Tile TRN2 kernel for nn_LocalTransformerBlock.

Sharding: pure data-parallel - batch B=8, one batch element per NeuronCore.

Per-core structure (4096 tokens, 32 row-tiles of 128):
  Phase A (all 32 tiles): LN1 + QKV + qk-l2norm/rope + windowed attention +
    out-proj + residual -> x2 kept resident in SBUF.
  Phase B (all 32 tiles): LN2 + FFN (transposed FF1 so gelu emits FF2's lhsT
    directly) + residual -> DRAM.
The phase split keeps the scalar engine on a single activation table per
phase (Exp in A, Gelu in B): 2 table loads total instead of 4 per tile.

Engine balance choices:
  - all rsqrt (LN + qk l2norm) via Newton iterations on DVE (bit-trick seed),
    no Sqrt activations;
  - q's l2norm scale is folded into the softmax exp's per-partition scale;
    k's is applied by per-head Copy-scale activations;
  - the causal mask enters as an extra PE matmul (NEG*tril^T @ I) accumulated
    into the S psum region, so exp's accum_out gives valid row sums;
  - matmuls in bf16/f16 (full PE rate at any free size), out-proj in f32r
    with 2-head-packed stationary operands;
  - psum->sbuf copies spread across Act (f16/bf16) and DVE (f16 2x);
    gpsimd/Q7 is avoided entirely (it wedges the device in this kernel's
    pipelined context, though isolated Q7 ops work).
"""
import numpy as np
from contextlib import ExitStack

import concourse.bass as bass
import concourse.bacc as bacc
import concourse.tile as tile
from concourse import masks as cmasks
from concourse import mybir
from concourse.bass_utils import run_bass_kernel_spmd

DIM = 512
HEADS = 8
DHEAD = 64
WIN = 128
NTOK = 4096
NT = NTOK // WIN          # 32 row tiles
B = 8
QK_SCALE = 8.0
NEG = -30000.0
RSQRT_MAGIC = 0x5F3759DF

F32 = mybir.dt.float32
F32R = mybir.dt.float32r
I32 = mybir.dt.int32
BF16 = mybir.dt.bfloat16
F16 = mybir.dt.float16
AF = mybir.ActivationFunctionType
ALU = mybir.AluOpType


def _bc(ap, dims):
    """Rebuild an AP with explicit [step, count] dims (for broadcasts)."""
    return bass.AP(tensor=ap.tensor, offset=ap.offset, ap=dims)


def build_program(has_qkv_bias, has_ff_bias, has_out_bias):
    nc = bacc.Bacc()

    x_d = nc.declare_dram_parameter("x", [NTOK, DIM], F32, isOutput=False)
    wqkvT_d = nc.declare_dram_parameter("wqkvT", [DIM, 3 * DIM], BF16, isOutput=False)
    woutT_d = nc.declare_dram_parameter("woutT", [DIM, DIM], F32R, isOutput=False)
    wff1T_d = nc.declare_dram_parameter("wff1T", [DIM, 4 * DIM], BF16, isOutput=False)
    wff2T_d = nc.declare_dram_parameter("wff2T", [4 * DIM, DIM], BF16, isOutput=False)
    rope_d = nc.declare_dram_parameter("rope", [NTOK, 4 * DHEAD], F32, isOutput=False)
    trineg_d = nc.declare_dram_parameter("trineg", [WIN, WIN], F16, isOutput=False)
    bias_d = None
    if has_qkv_bias or has_ff_bias or has_out_bias:
        bias_d = nc.declare_dram_parameter("biases", [3 * DIM + DIM + 4 * DIM], F32,
                                           isOutput=False)
    out_d = nc.declare_dram_parameter("out", [NTOK, DIM], F32, isOutput=True)

    with ExitStack() as ctx:
        tc = ctx.enter_context(tile.TileContext(nc))
        consts = ctx.enter_context(tc.tile_pool(name="consts", bufs=1))
        io = ctx.enter_context(tc.tile_pool(name="io", bufs=3))
        work = ctx.enter_context(tc.tile_pool(name="work", bufs=2))
        apool = ctx.enter_context(tc.tile_pool(name="apool", bufs=5))
        atp = ctx.enter_context(tc.tile_pool(name="atp", bufs=3))
        small = ctx.enter_context(tc.tile_pool(name="small", bufs=4))
        psQK = ctx.enter_context(tc.tile_pool(name="psQK", bufs=1, space="PSUM"))
        psS = ctx.enter_context(tc.tile_pool(name="psS", bufs=2, space="PSUM"))
        psT = ctx.enter_context(tc.tile_pool(name="psT", bufs=2, space="PSUM"))
        psP = ctx.enter_context(tc.tile_pool(name="psP", bufs=2, space="PSUM"))

        # ---- resident constants ----
        wq_sb = consts.tile([128, 4, 3 * DIM], BF16)
        wo_sb = consts.tile([128, 4, DIM], F32R)
        wf1_sb = consts.tile([128, 4, 4 * DIM], BF16)
        wf2_sb = consts.tile([128, 16, DIM], BF16)
        for k in range(4):
            nc.sync.dma_start(out=wq_sb[:, k, :], in_=wqkvT_d[k * 128:(k + 1) * 128, :])
            nc.sync.dma_start(out=wf1_sb[:, k, :], in_=wff1T_d[k * 128:(k + 1) * 128, :])
            nc.sync.dma_start(out=wo_sb[:, k, :], in_=woutT_d[k * 128:(k + 1) * 128, :])
        for k in range(16):
            nc.sync.dma_start(out=wf2_sb[:, k, :], in_=wff2T_d[k * 128:(k + 1) * 128, :])
        eye_bf = consts.tile([128, 128], BF16)
        cmasks.make_identity(nc, eye_bf[:, :])
        eye_f16 = consts.tile([128, 128], F16)
        cmasks.make_identity(nc, eye_f16[:, :])
        trineg_sb = consts.tile([128, WIN], F16)
        nc.sync.dma_start(out=trineg_sb, in_=trineg_d[:, :])
        bias_sb = None
        bias_ffT = None
        if bias_d is not None:
            bias_sb = consts.tile([128, 4 * DIM], F32)
            nc.sync.dma_start(out=bias_sb,
                              in_=_bc(bias_d[0:4 * DIM], [[0, 128], [1, 4 * DIM]]))
            bias_ffT = consts.tile([128, 16], F32)
            nc.sync.dma_start(out=bias_ffT,
                              in_=bias_d[4 * DIM:].rearrange("(c p) -> p c", p=128))

        # k/v rings: slot t%2 holds tile t's transposed keys / values.
        kslot = [consts.tile([128, 4, WIN], F16, name=f"kslot{i}") for i in range(2)]
        vslot = [consts.tile([128, DIM], F16, name=f"vslot{i}") for i in range(2)]

        # phase-A -> phase-B carry: x2 (attn residual out) for all tiles
        x2s = [consts.tile([128, DIM], F32, name=f"x2_{t}") for t in range(NT)]

        def newton_rsqrt(dst, v, n, tag):
            """dst[128, n] f32 = 1/sqrt(v); v > 0 (bit-trick seed + 2 iters)."""
            t1 = small.tile([128, n], F32, name=f"nr_{tag}")
            t1i = t1.bitcast(I32)
            di = dst.bitcast(I32)
            vi = v.bitcast(I32)
            nc.vector.tensor_scalar(out=t1i, in0=vi, scalar1=1, scalar2=None,
                                    op0=ALU.logical_shift_right)
            nc.vector.tensor_scalar(out=di, in0=t1i, scalar1=-1, scalar2=RSQRT_MAGIC,
                                    op0=ALU.mult, op1=ALU.add)
            for _ in range(2):
                nc.vector.tensor_mul(t1, dst, dst)
                nc.vector.tensor_mul(t1, t1, v)
                nc.vector.tensor_scalar(out=t1, in0=t1, scalar1=-0.5, scalar2=1.5,
                                        op0=ALU.mult, op1=ALU.add)
                nc.vector.tensor_mul(dst, dst, t1)

        def layernorm_stats(src, tag):
            """-> (mv [128,2] mean/var, rstd [128,1]) on DVE only."""
            st = small.tile([128, nc.vector.BN_STATS_DIM], F32, name=f"st_{tag}")
            nc.vector.bn_stats(st, src)
            mv = small.tile([128, nc.vector.BN_AGGR_DIM], F32, name=f"mv_{tag}")
            nc.vector.bn_aggr(mv, st)
            rstd = small.tile([128, 1], F32, name=f"rstd_{tag}")
            newton_rsqrt(rstd, mv[:, 1:2], 1, tag)
            return mv, rstd

        def stage_a(t):
            cur, prv = t % 2, (t + 1) % 2

            x_t = io.tile([128, DIM], F32, name="x_t")
            nc.sync.dma_start(out=x_t, in_=x_d[t * 128:(t + 1) * 128, :])
            rope_t = io.tile([128, 4 * DHEAD], F32, name="rope_t")
            nc.sync.dma_start(out=rope_t, in_=rope_d[t * 128:(t + 1) * 128, :])

            # ---- LN1 (gains folded into weights host-side) ----
            mv, rstd = layernorm_stats(x_t, "ln1")
            h = work.tile([128, DIM], BF16, name="h_x")
            nc.vector.tensor_scalar(out=h, in0=x_t, scalar1=mv[:, 0:1],
                                    scalar2=rstd, op0=ALU.subtract, op1=ALU.mult)

            # ---- hT (bf16 transposes; copies on Act) ----
            hT = work.tile([128, 4, 128], BF16, name="hT")
            for k in range(4):
                pt = psT.tile([128, 512], F32, name="ptA", tag="ptA")
                ptb = pt[:, 0:64].bitcast(BF16)
                nc.tensor.transpose(ptb, h[:, k * 128:(k + 1) * 128], eye_bf)
                nc.scalar.copy(out=hT[:, k, :], in_=ptb)

            # ---- QKV (bf16) ----
            pm_q = psQK.tile([128, 512], F32, name="pm_q", tag="pm_q")
            pm_k = psQK.tile([128, 512], F32, name="pm_k", tag="pm_k")
            pm_v = psT.tile([128, 512], F32, name="ptA", tag="ptA")
            for c, pm in ((0, pm_q), (1, pm_k), (2, pm_v)):
                for k in range(4):
                    nc.tensor.matmul(pm, hT[:, k, :],
                                     wq_sb[:, k, c * 512:(c + 1) * 512],
                                     start=(k == 0), stop=(k == 3))
                if has_qkv_bias:
                    nc.vector.tensor_add(pm, pm, bias_sb[:, c * 512:(c + 1) * 512])
            nc.scalar.copy(out=vslot[cur], in_=pm_v)

            # gpsimd (Q7) cannot touch PSUM: stage raw q/k to SBUF via Act
            q_sb = work.tile([128, DIM], F32, name="q_sb")
            nc.scalar.copy(out=q_sb, in_=pm_q)
            k_sb = work.tile([128, DIM], F32, name="k_sb")
            nc.scalar.copy(out=k_sb, in_=pm_k)
            q3 = q_sb.rearrange("p (h d) -> p h d", h=HEADS)
            k3 = k_sb.rearrange("p (h d) -> p h d", h=HEADS)

            # ---- rope (q/k scales baked into tables host-side) ----
            qcos = rope_t[:, 0:DHEAD]
            qsin = rope_t[:, DHEAD:2 * DHEAD]
            kcos = rope_t[:, 2 * DHEAD:3 * DHEAD]
            ksin = rope_t[:, 3 * DHEAD:4 * DHEAD]

            def rope_mix(src3, cos, sin, dst, tag):
                cosB = _bc(cos, [cos.ap[0], [0, HEADS], cos.ap[1]])
                sinLoB = _bc(sin[:, 0:32], [sin.ap[0], [0, HEADS], [1, 32]])
                sinHiB = _bc(sin[:, 32:64], [sin.ap[0], [0, HEADS], [1, 32]])
                rot = work.tile([128, DIM], F32, name="rot", tag="rot")
                rot3 = rot.rearrange("p (h d) -> p h d", h=HEADS)
                nc.vector.tensor_mul(rot3[:, :, 0:32], src3[:, :, 32:64], sinLoB)
                nc.vector.tensor_mul(rot3[:, :, 32:64], src3[:, :, 0:32], sinHiB)
                cc = work.tile([128, DIM], F32, name="cc", tag="cc")
                cc3 = cc.rearrange("p (h d) -> p h d", h=HEADS)
                nc.vector.tensor_mul(cc3, src3, cosB)
                nc.vector.tensor_add(dst, cc, rot)

            qr = work.tile([128, DIM], F16, name="qr")
            rope_mix(q3, qcos, qsin, qr, "q")
            kfull = work.tile([128, DIM], F32, name="kfull")
            rope_mix(k3, kcos, ksin, kfull, "k")

            # ---- per-head sumsq AFTER rope (rope is a norm-preserving
            # rotation, and q/k_scale are constant per spec), so TTR reads
            # SBUF (psum allows only one operand read per op). q side is
            # scaled by 1/DHEAD: rn_q = QK_SCALE*rsqrt(sumsq) since
            # QK_SCALE = sqrt(DHEAD).
            qr3v = qr.rearrange("p (h d) -> p h d", h=HEADS)
            kf3 = kfull.rearrange("p (h d) -> p h d", h=HEADS)
            sq = small.tile([128, 2 * HEADS], F32, name="sq")
            scr = work.tile([128, DIM], F32, name="scr", tag="scr")
            scr3 = scr.rearrange("p (h d) -> p h d", h=HEADS)
            nc.vector.tensor_mul(scr3, qr3v, qr3v)
            nc.vector.tensor_reduce(out=sq[:, 0:HEADS], in_=scr3,
                                    axis=mybir.AxisListType.X, op=ALU.add)
            nc.vector.tensor_mul(scr3, kf3, kf3)
            nc.vector.tensor_reduce(out=sq[:, HEADS:2 * HEADS], in_=scr3,
                                    axis=mybir.AxisListType.X, op=ALU.add)
            # q half scaled by 1/DHEAD: rn_q = QK_SCALE*rsqrt(sumsq)
            nc.vector.tensor_scalar_mul(sq[:, 0:HEADS], sq[:, 0:HEADS],
                                        1.0 / DHEAD)
            rn = small.tile([128, 2 * HEADS], F32, name="rn")
            newton_rsqrt(rn, sq, 2 * HEADS, "rn")

            # k's l2norm scale in one DVE pass (broadcast rn_k over dhead)
            kr = work.tile([128, DIM], F16, name="kr")
            kr3 = kr.rearrange("p (h d) -> p h d", h=HEADS)
            rnk = rn[:, HEADS:2 * HEADS]
            rnkB = _bc(rnk, rnk.ap + [[0, DHEAD]])
            nc.vector.tensor_mul(kr3, kf3, rnkB)

            # ---- joint head-pair transposes of q', k' (f16) ----
            qT = work.tile([128, 4, 128], F16, name="qT")
            for p in range(4):
                pt = psT.tile([128, 512], F32, name="ptA", tag="ptA")
                ptf = pt[:, 0:64].bitcast(F16)
                nc.tensor.transpose(ptf, qr[:, p * 128:(p + 1) * 128], eye_f16)
                nc.scalar.copy(out=qT[:, p, :], in_=ptf)
                pt2 = psT.tile([128, 512], F32, name="ptA", tag="ptA")
                ptf2 = pt2[:, 0:64].bitcast(F16)
                nc.tensor.transpose(ptf2, kr[:, p * 128:(p + 1) * 128], eye_f16)
                nc.scalar.copy(out=kslot[cur][:, p, :], in_=ptf2)

            # ---- S + masked exp (per head pair) ----
            rs8 = small.tile([128, HEADS], F32, name="rs8")
            As = []
            for p in range(4):
                pm = psS.tile([128, 512], F32, name="pmS", tag="pmS")
                A = apool.tile([128, 512], F16, name=f"A{p}", tag=f"A{p}")
                for half in range(2):
                    hd = 2 * p + half
                    base = 256 * half
                    lo, hi = 64 * half, 64 * half + 64
                    lhs_q = qT[lo:hi, p, :]
                    # additive causal mask for the diagonal block, via PE
                    nc.tensor.matmul(pm[:, base + 128:base + 256], trineg_sb,
                                     eye_f16, start=True, stop=False)
                    nc.tensor.matmul(pm[:, base + 128:base + 256], lhs_q,
                                     kslot[cur][lo:hi, p, :],
                                     start=False, stop=True)
                    if t > 0:
                        nc.tensor.matmul(pm[:, base:base + 128], lhs_q,
                                         kslot[prv][lo:hi, p, :],
                                         start=True, stop=True)
                        sl = slice(base, base + 256)
                    else:
                        sl = slice(base + 128, base + 256)
                    # exp with q's l2norm scale folded in; accum -> row sums
                    nc.scalar.activation(out=A[:, sl], in_=pm[:, sl], func=AF.Exp,
                                         scale=rn[:, hd:hd + 1],
                                         accum_out=rs8[:, hd:hd + 1])
                As.append(A)
            ri8 = small.tile([128, HEADS], F32, name="ri8")
            nc.vector.reciprocal(ri8, rs8)

            # ---- normalize A, transpose, AV, pack PT ----
            PTsb = work.tile([128, 4, 128], F32R, name="PTsb")
            for p in range(4):
                A = As[p]
                for half in range(2):
                    hd = 2 * p + half
                    base = 256 * half
                    sl = slice(base, base + 256) if t > 0 else \
                        slice(base + 128, base + 256)
                    nc.vector.tensor_scalar_mul(A[:, sl], A[:, sl],
                                                ri8[:, hd:hd + 1])
                pp = psP.tile([128, 512], F32, name="ppP", tag="ppP")
                for half in range(2):
                    hd = 2 * p + half
                    base = 256 * half
                    lo, hi = 64 * half, 64 * half + 64
                    AT = atp.tile([128, 2 * WIN], F16, name="AT")
                    if t > 0:
                        ptp = psT.tile([128, 512], F32, name="ptA", tag="ptA")
                        ptpf = ptp[:, 0:64].bitcast(F16)
                        nc.tensor.transpose(ptpf, A[:, base:base + 128], eye_f16)
                        nc.scalar.copy(out=AT[:, 0:128], in_=ptpf)
                    ptc = psT.tile([128, 512], F32, name="ptA", tag="ptA")
                    ptcf = ptc[:, 0:64].bitcast(F16)
                    nc.tensor.transpose(ptcf, A[:, base + 128:base + 256], eye_f16)
                    nc.scalar.copy(out=AT[:, 128:256], in_=ptcf)
                    vsl = slice(hd * DHEAD, (hd + 1) * DHEAD)
                    if t > 0:
                        nc.tensor.matmul(pp[lo:hi, 0:128], vslot[prv][:, vsl],
                                         AT[:, 0:128], start=True, stop=False)
                        nc.tensor.matmul(pp[lo:hi, 0:128], vslot[cur][:, vsl],
                                         AT[:, 128:256], start=False, stop=True)
                    else:
                        nc.tensor.matmul(pp[lo:hi, 0:128], vslot[cur][:, vsl],
                                         AT[:, 128:256], start=True, stop=True)
                nc.scalar.copy(out=PTsb[:, p, :], in_=pp[:, 0:128])

            # ---- output projection + residual (f32r) ----
            py = psP.tile([128, 512], F32, name="ppP", tag="ppP")
            for p in range(4):
                nc.tensor.matmul(py, PTsb[:, p, :], wo_sb[:, p, :],
                                 start=(p == 0), stop=(p == 3))
            if has_out_bias:
                nc.vector.tensor_add(py, py, bias_sb[:, 3 * DIM:4 * DIM])
            nc.vector.tensor_add(x2s[t], x_t, py)

        def stage_b(t):
            x2 = x2s[t]

            # ---- LN2 + FFN ----
            mv2, rstd2 = layernorm_stats(x2, "ln2")
            h2 = work.tile([128, DIM], BF16, name="h2")
            nc.vector.tensor_scalar(out=h2, in0=x2, scalar1=mv2[:, 0:1],
                                    scalar2=rstd2, op0=ALU.subtract, op1=ALU.mult)
            h2T = work.tile([128, 4, 128], BF16, name="h2T")
            for k in range(4):
                pt = psT.tile([128, 512], F32, name="ptA", tag="ptA")
                ptb = pt[:, 0:64].bitcast(BF16)
                nc.tensor.transpose(ptb, h2[:, k * 128:(k + 1) * 128], eye_bf)
                nc.scalar.copy(out=h2T[:, k, :], in_=ptb)

            # FF1 transposed: psum holds gT chunks; gelu writes FF2's lhsT
            g_sb = work.tile([128, 16, 128], BF16, name="g_sb")
            py2 = psP.tile([128, 512], F32, name="ppP", tag="ppP")
            for c in range(16):
                pg = psT.tile([128, 512], F32, name="ptA", tag="ptA")
                for k in range(4):
                    nc.tensor.matmul(pg[:, 0:128],
                                     wf1_sb[:, k, c * 128:(c + 1) * 128],
                                     h2T[:, k, :], start=(k == 0), stop=(k == 3))
                if has_ff_bias:
                    nc.scalar.activation(out=g_sb[:, c, :], in_=pg[:, 0:128],
                                         func=AF.Gelu, bias=bias_ffT[:, c:c + 1])
                else:
                    nc.scalar.activation(out=g_sb[:, c, :], in_=pg[:, 0:128],
                                         func=AF.Gelu)
                nc.tensor.matmul(py2, g_sb[:, c, :], wf2_sb[:, c, :],
                                 start=(c == 0), stop=(c == 15))
            out_t = work.tile([128, DIM], F32, name="out_t")
            nc.vector.tensor_add(out_t, x2, py2)
            nc.sync.dma_start(out=out_d[t * 128:(t + 1) * 128, :], in_=out_t)

        for t in range(NT):
            stage_a(t)
        for t in range(NT):
            stage_b(t)

    nc.compile()
    return nc


_CACHE = {}


def prepare(x, w_qkv, q_scale, k_scale, w_out, b_out, ln1_g, ln1_b,
            ff_ln_g, ff_ln_b, w_ff1, w_ff2):
    x = np.asarray(x, np.float32)

    _bf = mybir.dt.np(BF16)
    _f16 = np.float16

    # ---- host-side folding ----
    ln1_g = np.asarray(ln1_g, np.float32)
    ln1_b = np.asarray(ln1_b, np.float32)
    ff_ln_g = np.asarray(ff_ln_g, np.float32)
    ff_ln_b = np.asarray(ff_ln_b, np.float32)
    w_qkv = np.asarray(w_qkv, np.float32)
    w_ff1 = np.asarray(w_ff1, np.float32)
    wqkvT = np.ascontiguousarray((w_qkv * ln1_g[None, :]).T).astype(_bf)
    woutT = np.ascontiguousarray(np.asarray(w_out, np.float32).T)     # (512,512)
    wff1T = np.ascontiguousarray((w_ff1 * ff_ln_g[None, :]).T).astype(_bf)
    wff2T = np.ascontiguousarray(np.asarray(w_ff2, np.float32).T).astype(_bf)
    bias_qkv = w_qkv @ ln1_b                                          # (1536,)
    bias_ff = w_ff1 @ ff_ln_b                                         # (2048,)
    b_out = np.asarray(b_out, np.float32)
    has_qkv_bias = bool(np.any(bias_qkv))
    has_ff_bias = bool(np.any(bias_ff))
    has_out_bias = bool(np.any(b_out))
    biases = np.concatenate([bias_qkv, b_out, bias_ff]).astype(np.float32)

    # rope tables with q/k scales and the 8.0 QK scale baked in
    pos = np.arange(NTOK, dtype=np.float32)
    inv_freq = 1.0 / (10000.0 ** (np.arange(0, DHEAD, 2, dtype=np.float32) / DHEAD))
    freqs = pos[:, None] * inv_freq
    emb = np.concatenate([freqs, freqs], axis=-1)                     # (NTOK, 64)
    cos, sin = np.cos(emb), np.sin(emb)
    qs = np.asarray(q_scale, np.float32)
    ks = np.asarray(k_scale, np.float32)
    rp = np.concatenate([qs[32:], qs[:32]])                           # rotperm
    kp = np.concatenate([ks[32:], ks[:32]])
    sgn = np.concatenate([-np.ones(32, np.float32), np.ones(32, np.float32)])
    # No QK_SCALE here: the device computes rn_q = rsqrt(sumsq(q_roped)/DHEAD)
    # = QK_SCALE * rsqrt(sumsq) (QK_SCALE^2 == DHEAD) and applies it as the
    # exp's per-partition scale.
    qcos = cos * qs[None, :]
    qsin = sin * rp[None, :] * sgn[None, :]
    kcos = cos * ks[None, :]
    ksin = sin * kp[None, :] * sgn[None, :]
    rope = np.concatenate([qcos, qsin, kcos, ksin], axis=1).astype(np.float32)

    # additive causal mask as matmul stationary operand: psum gets
    # trineg.T = NEG * strict_upper (invalid j > i of the diagonal block)
    trineg = (np.tril(np.full((WIN, WIN), NEG, np.float32), k=-1)).astype(_f16)

    key = (has_qkv_bias, has_ff_bias, has_out_bias)
    if key not in _CACHE:
        _CACHE[key] = build_program(*key)
    nc = _CACHE[key]

    shared = dict(wqkvT=wqkvT, woutT=woutT, wff1T=wff1T, wff2T=wff2T,
                  rope=rope, trineg=trineg)
    if key != (False, False, False):
        shared["biases"] = biases
    in_maps = [dict(x=np.ascontiguousarray(x[i]), **shared) for i in range(B)]
    return nc, in_maps


def kernel(x, w_qkv, q_scale, k_scale, w_out, b_out, ln1_g, ln1_b,
           ff_ln_g, ff_ln_b, w_ff1, w_ff2, **run_kwargs):
    nc, in_maps = prepare(x, w_qkv, q_scale, k_scale, w_out, b_out, ln1_g,
                          ln1_b, ff_ln_g, ff_ln_b, w_ff1, w_ff2)
    res = run_bass_kernel_spmd(nc, in_maps, list(range(B)), **run_kwargs)
    out = np.stack([res.results[i]["out"] for i in range(B)]).astype(np.float32)
    if run_kwargs:
        return out, res
    return out


# revision 18
# speedup vs baseline: 1.4424x; 1.3198x over previous
"""Bass/Tile TRN2 kernel for nn_LocalTransformerBlock.

Sharding: pure data-parallel - batch B=8, one batch element per NeuronCore.

Per-core structure (4096 tokens, 32 row-tiles of 128):
  Phase A (all 32 tiles): LN1 + QKV + qk-l2norm/rope + windowed attention +
    out-proj + residual -> x2 kept resident in SBUF (bf16).
  Phase B (all 32 tiles): LN2 + FFN (transposed FF1 so gelu emits FF2's lhsT
    directly) + residual -> DRAM.
The phase split keeps the scalar engine on a single activation table per
phase (Exp in A, Gelu in B): 2 table loads total instead of 4 per tile.

Each phase is software-pipelined at EMISSION time: a tile's work is split
into sub-stages (sa1..sa4 / sb1..sb2) and emission interleaves sub-stages of
consecutive tiles. The engines execute their queues in order, so without
this interleave a tile's ~100-op dependency chain serializes the whole
machine (measured 1.3ms); with it, every engine has independent work queued
between dependent ops.

Engine balance choices:
  - all rsqrt (LN + qk l2norm) via Newton iterations on DVE (bit-trick
    seed), no Sqrt activations;
  - q's l2norm scale is folded into the softmax exp's per-partition scale
    (sumsq taken post-rope: rope is a norm-preserving rotation);
  - the causal mask enters as an extra PE matmul (NEG*tril^T @ I)
    accumulated into the S psum region, so exp's accum_out gives row sums;
  - matmuls in bf16/f16 (full PE rate at any free size), out-proj in f32r
    with 2-head-packed stationary operands;
  - gpsimd/Q7 and tensor_tensor_reduce are avoided entirely (both wedge the
    device in this kernel's context, though isolated ops work).
"""
import numpy as np
from contextlib import ExitStack

import concourse.bass as bass
import concourse.bacc as bacc
import concourse.tile as tile
from concourse import masks as cmasks
from concourse import mybir
from concourse.bass_utils import run_bass_kernel_spmd

DIM = 512
HEADS = 8
DHEAD = 64
WIN = 128
NTOK = 4096
NT = NTOK // WIN          # 32 row tiles
B = 8
QK_SCALE = 8.0
NEG = -30000.0
RSQRT_MAGIC = 0x5F3759DF

F32 = mybir.dt.float32
F32R = mybir.dt.float32r
I32 = mybir.dt.int32
BF16 = mybir.dt.bfloat16
F16 = mybir.dt.float16
AF = mybir.ActivationFunctionType
ALU = mybir.AluOpType


def _bc(ap, dims):
    """Rebuild an AP with explicit [step, count] dims (for broadcasts)."""
    return bass.AP(tensor=ap.tensor, offset=ap.offset, ap=dims)


def build_program(has_qkv_bias, has_ff_bias, has_out_bias):
    nc = bacc.Bacc()

    x_d = nc.declare_dram_parameter("x", [NTOK, DIM], F32, isOutput=False)
    wqkvT_d = nc.declare_dram_parameter("wqkvT", [DIM, 3 * DIM], BF16, isOutput=False)
    woutT_d = nc.declare_dram_parameter("woutT", [DIM, DIM], F32R, isOutput=False)
    wff1T_d = nc.declare_dram_parameter("wff1T", [DIM, 4 * DIM], BF16, isOutput=False)
    wff2T_d = nc.declare_dram_parameter("wff2T", [4 * DIM, DIM], BF16, isOutput=False)
    rope_d = nc.declare_dram_parameter("rope", [NTOK, 4 * DHEAD], F32, isOutput=False)
    trineg_d = nc.declare_dram_parameter("trineg", [WIN, WIN], F16, isOutput=False)
    bias_d = None
    if has_qkv_bias or has_ff_bias or has_out_bias:
        bias_d = nc.declare_dram_parameter("biases", [3 * DIM + DIM + 4 * DIM], F32,
                                           isOutput=False)
    out_d = nc.declare_dram_parameter("out", [NTOK, DIM], F32, isOutput=True)

    with ExitStack() as ctx:
        tc = ctx.enter_context(tile.TileContext(nc))
        consts = ctx.enter_context(tc.tile_pool(name="consts", bufs=1))
        io = ctx.enter_context(tc.tile_pool(name="io", bufs=5))
        work = ctx.enter_context(tc.tile_pool(name="work", bufs=2))
        wq3 = ctx.enter_context(tc.tile_pool(name="wq3", bufs=3))
        apool = ctx.enter_context(tc.tile_pool(name="apool", bufs=2))
        atp = ctx.enter_context(tc.tile_pool(name="atp", bufs=3))
        small = ctx.enter_context(tc.tile_pool(name="small", bufs=4))
        psQK = ctx.enter_context(tc.tile_pool(name="psQK", bufs=1, space="PSUM"))
        psS = ctx.enter_context(tc.tile_pool(name="psS", bufs=2, space="PSUM"))
        psT = ctx.enter_context(tc.tile_pool(name="psT", bufs=2, space="PSUM"))
        psP = ctx.enter_context(tc.tile_pool(name="psP", bufs=2, space="PSUM"))

        # ---- resident constants ----
        wq_sb = consts.tile([128, 4, 3 * DIM], BF16)
        wo_sb = consts.tile([128, 4, DIM], F32R)
        wf1_sb = consts.tile([128, 4, 4 * DIM], BF16)
        wf2_sb = consts.tile([128, 16, DIM], BF16)
        for k in range(4):
            nc.sync.dma_start(out=wq_sb[:, k, :], in_=wqkvT_d[k * 128:(k + 1) * 128, :])
            nc.sync.dma_start(out=wf1_sb[:, k, :], in_=wff1T_d[k * 128:(k + 1) * 128, :])
            nc.sync.dma_start(out=wo_sb[:, k, :], in_=woutT_d[k * 128:(k + 1) * 128, :])
        for k in range(16):
            nc.sync.dma_start(out=wf2_sb[:, k, :], in_=wff2T_d[k * 128:(k + 1) * 128, :])
        eye_bf = consts.tile([128, 128], BF16)
        cmasks.make_identity(nc, eye_bf[:, :])
        eye_f16 = consts.tile([128, 128], F16)
        cmasks.make_identity(nc, eye_f16[:, :])
        trineg_sb = consts.tile([128, WIN], F16)
        nc.sync.dma_start(out=trineg_sb, in_=trineg_d[:, :])
        bias_sb = None
        bias_ffT = None
        if bias_d is not None:
            bias_sb = consts.tile([128, 4 * DIM], F32)
            nc.sync.dma_start(out=bias_sb,
                              in_=_bc(bias_d[0:4 * DIM], [[0, 128], [1, 4 * DIM]]))
            bias_ffT = consts.tile([128, 16], F32)
            nc.sync.dma_start(out=bias_ffT,
                              in_=bias_d[4 * DIM:].rearrange("(c p) -> p c", p=128))

        # k/v rings. vslot[t-1] is read by sa4(t) at pipeline round t+3 and
        # slot t%N is rewritten by sa1(t+N) at round t+N: need N >= 5.
        NSLOT = 5
        kslot = [consts.tile([128, 4, WIN], F16, name=f"kslot{i}")
                 for i in range(NSLOT)]
        vslot = [consts.tile([128, DIM], F16, name=f"vslot{i}")
                 for i in range(NSLOT)]

        # phase-A -> phase-B carry: x2 (attn residual out) for all tiles
        x2s = [consts.tile([128, DIM], BF16, name=f"x2_{t}") for t in range(NT)]

        def newton_rsqrt(dst, v, n, tag):
            """dst[128, n] f32 = 1/sqrt(v); v > 0 (bit-trick seed + 2 iters)."""
            t1 = small.tile([128, n], F32, name=f"nr_{tag}")
            t1i = t1.bitcast(I32)
            di = dst.bitcast(I32)
            vi = v.bitcast(I32)
            nc.vector.tensor_scalar(out=t1i, in0=vi, scalar1=1, scalar2=None,
                                    op0=ALU.logical_shift_right)
            nc.vector.tensor_scalar(out=di, in0=t1i, scalar1=-1, scalar2=RSQRT_MAGIC,
                                    op0=ALU.mult, op1=ALU.add)
            for _ in range(2):
                nc.vector.tensor_mul(t1, dst, dst)
                nc.vector.tensor_mul(t1, t1, v)
                nc.vector.tensor_scalar(out=t1, in0=t1, scalar1=-0.5, scalar2=1.5,
                                        op0=ALU.mult, op1=ALU.add)
                nc.vector.tensor_mul(dst, dst, t1)

        def layernorm_stats(src, tag):
            """-> (mv [128,2] mean/var, rstd [128,1]) on DVE only."""
            st = small.tile([128, nc.vector.BN_STATS_DIM], F32, name=f"st_{tag}")
            nc.vector.bn_stats(st, src)
            mv = small.tile([128, nc.vector.BN_AGGR_DIM], F32, name=f"mv_{tag}")
            nc.vector.bn_aggr(mv, st)
            rstd = small.tile([128, 1], F32, name=f"rstd_{tag}")
            newton_rsqrt(rstd, mv[:, 1:2], 1, tag)
            return mv, rstd

        A_state = {}

        def sa1(t):
            """DMA + LN1 + hT + QKV + psum->sbuf staging."""
            st = {}
            x_t = io.tile([128, DIM], F32, name="x_t")
            nc.sync.dma_start(out=x_t, in_=x_d[t * 128:(t + 1) * 128, :])
            rope_t = io.tile([128, 4 * DHEAD], F32, name="rope_t")
            nc.sync.dma_start(out=rope_t, in_=rope_d[t * 128:(t + 1) * 128, :])
            st["x_t"], st["rope_t"] = x_t, rope_t

            mv, rstd = layernorm_stats(x_t, "ln1")
            h = work.tile([128, DIM], BF16, name="h_x")
            nc.vector.tensor_scalar(out=h, in0=x_t, scalar1=mv[:, 0:1],
                                    scalar2=rstd, op0=ALU.subtract, op1=ALU.mult)

            hT = work.tile([128, 4, 128], BF16, name="hT")
            for k in range(4):
                pt = psT.tile([128, 512], F32, name="ptA", tag="ptA")
                ptb = pt[:, 0:64].bitcast(BF16)
                nc.tensor.transpose(ptb, h[:, k * 128:(k + 1) * 128], eye_bf)
                nc.scalar.copy(out=hT[:, k, :], in_=ptb)

            pm_q = psQK.tile([128, 512], F32, name="pm_q", tag="pm_q")
            pm_k = psQK.tile([128, 512], F32, name="pm_k", tag="pm_k")
            pm_v = psT.tile([128, 512], F32, name="ptA", tag="ptA")
            for c, pm in ((0, pm_q), (1, pm_k), (2, pm_v)):
                for k in range(4):
                    nc.tensor.matmul(pm, hT[:, k, :],
                                     wq_sb[:, k, c * 512:(c + 1) * 512],
                                     start=(k == 0), stop=(k == 3))
                if has_qkv_bias:
                    nc.vector.tensor_add(pm, pm, bias_sb[:, c * 512:(c + 1) * 512])
            nc.scalar.copy(out=vslot[t % NSLOT], in_=pm_v)
            q_sb = wq3.tile([128, DIM], F32, name="q_sb")
            nc.scalar.copy(out=q_sb, in_=pm_q)
            k_sb = wq3.tile([128, DIM], F32, name="k_sb")
            nc.scalar.copy(out=k_sb, in_=pm_k)
            st["q_sb"], st["k_sb"] = q_sb, k_sb
            A_state[t] = st

        def sa2(t):
            """rope + l2norm scales + q/k head-pair transposes."""
            st = A_state[t]
            rope_t = st.pop("rope_t")
            q_sb, k_sb = st.pop("q_sb"), st.pop("k_sb")
            q3 = q_sb.rearrange("p (h d) -> p h d", h=HEADS)
            k3 = k_sb.rearrange("p (h d) -> p h d", h=HEADS)
            qcos = rope_t[:, 0:DHEAD]
            qsin = rope_t[:, DHEAD:2 * DHEAD]
            kcos = rope_t[:, 2 * DHEAD:3 * DHEAD]
            ksin = rope_t[:, 3 * DHEAD:4 * DHEAD]

            def rope_mix(src3, cos, sin, dst):
                cosB = _bc(cos, [cos.ap[0], [0, HEADS], cos.ap[1]])
                sinLoB = _bc(sin[:, 0:32], [sin.ap[0], [0, HEADS], [1, 32]])
                sinHiB = _bc(sin[:, 32:64], [sin.ap[0], [0, HEADS], [1, 32]])
                rot = work.tile([128, DIM], F32, name="rot", tag="rot")
                rot3 = rot.rearrange("p (h d) -> p h d", h=HEADS)
                nc.vector.tensor_mul(rot3[:, :, 0:32], src3[:, :, 32:64], sinLoB)
                nc.vector.tensor_mul(rot3[:, :, 32:64], src3[:, :, 0:32], sinHiB)
                cc = work.tile([128, DIM], F32, name="cc", tag="cc")
                cc3 = cc.rearrange("p (h d) -> p h d", h=HEADS)
                nc.vector.tensor_mul(cc3, src3, cosB)
                nc.vector.tensor_add(dst, cc, rot)

            qr = work.tile([128, DIM], F16, name="qr")
            rope_mix(q3, qcos, qsin, qr)
            kfull = work.tile([128, DIM], F32, name="kfull")
            rope_mix(k3, kcos, ksin, kfull)

            # per-head sumsq AFTER rope (norm-preserving rotation; q/k_scale
            # constant per spec). q half gets 1/DHEAD so that
            # rn_q = QK_SCALE*rsqrt(sumsq)  (QK_SCALE^2 == DHEAD).
            qr3v = qr.rearrange("p (h d) -> p h d", h=HEADS)
            kf3 = kfull.rearrange("p (h d) -> p h d", h=HEADS)
            sq = small.tile([128, 2 * HEADS], F32, name="sq")
            scr = work.tile([128, DIM], F32, name="scr", tag="scr")
            scr3 = scr.rearrange("p (h d) -> p h d", h=HEADS)
            nc.vector.tensor_mul(scr3, qr3v, qr3v)
            nc.vector.tensor_reduce(out=sq[:, 0:HEADS], in_=scr3,
                                    axis=mybir.AxisListType.X, op=ALU.add)
            nc.vector.tensor_mul(scr3, kf3, kf3)
            nc.vector.tensor_reduce(out=sq[:, HEADS:2 * HEADS], in_=scr3,
                                    axis=mybir.AxisListType.X, op=ALU.add)
            nc.vector.tensor_scalar_mul(sq[:, 0:HEADS], sq[:, 0:HEADS],
                                        1.0 / DHEAD)
            rn = small.tile([128, 2 * HEADS], F32, name="rn")
            newton_rsqrt(rn, sq, 2 * HEADS, "rn")
            st["rn"] = rn

            # k's l2norm scale in one DVE pass (broadcast rn_k over dhead)
            kr = work.tile([128, DIM], F16, name="kr")
            kr3 = kr.rearrange("p (h d) -> p h d", h=HEADS)
            rnk = rn[:, HEADS:2 * HEADS]
            rnkB = _bc(rnk, rnk.ap + [[0, DHEAD]])
            nc.vector.tensor_mul(kr3, kf3, rnkB)

            qT = wq3.tile([128, 4, 128], F16, name="qT")
            for p in range(4):
                pt = psT.tile([128, 512], F32, name="ptA", tag="ptA")
                ptf = pt[:, 0:64].bitcast(F16)
                nc.tensor.transpose(ptf, qr[:, p * 128:(p + 1) * 128], eye_f16)
                nc.scalar.copy(out=qT[:, p, :], in_=ptf)
                pt2 = psT.tile([128, 512], F32, name="ptA", tag="ptA")
                ptf2 = pt2[:, 0:64].bitcast(F16)
                nc.tensor.transpose(ptf2, kr[:, p * 128:(p + 1) * 128], eye_f16)
                nc.scalar.copy(out=kslot[t % NSLOT][:, p, :], in_=ptf2)
            st["qT"] = qT

        def sa3(t):
            """S matmuls + masked exp + per-pair normalize."""
            st = A_state[t]
            qT, rn = st.pop("qT"), st.pop("rn")
            kcur = kslot[t % NSLOT]
            kprv = kslot[(t - 1) % NSLOT]
            As = []
            for p in range(4):
                pm = psS.tile([128, 512], F32, name="pmS", tag="pmS")
                A = apool.tile([128, 512], F16, name=f"A{p}", tag=f"A{p}")
                rs2 = small.tile([128, 2], F32, name=f"rs{p}", tag=f"rs{p}")
                for half in range(2):
                    hd = 2 * p + half
                    base = 256 * half
                    lo, hi = 64 * half, 64 * half + 64
                    lhs_q = qT[lo:hi, p, :]
                    # additive causal mask for the diagonal block, via PE
                    nc.tensor.matmul(pm[:, base + 128:base + 256], trineg_sb,
                                     eye_f16, start=True, stop=False)
                    nc.tensor.matmul(pm[:, base + 128:base + 256], lhs_q,
                                     kcur[lo:hi, p, :], start=False, stop=True)
                    if t > 0:
                        nc.tensor.matmul(pm[:, base:base + 128], lhs_q,
                                         kprv[lo:hi, p, :], start=True, stop=True)
                        sl = slice(base, base + 256)
                    else:
                        sl = slice(base + 128, base + 256)
                    # exp with q's l2norm scale folded in; accum -> row sums
                    nc.scalar.activation(out=A[:, sl], in_=pm[:, sl], func=AF.Exp,
                                         scale=rn[:, hd:hd + 1],
                                         accum_out=rs2[:, half:half + 1])
                ri2 = small.tile([128, 2], F32, name=f"ri{p}", tag=f"ri{p}")
                nc.vector.reciprocal(ri2, rs2)
                for half in range(2):
                    base = 256 * half
                    sl = slice(base, base + 256) if t > 0 else \
                        slice(base + 128, base + 256)
                    nc.vector.tensor_scalar_mul(A[:, sl], A[:, sl],
                                                ri2[:, half:half + 1])
                As.append(A)
            st["As"] = As

        def sa4(t):
            """A transposes + AV + out-projection + residual."""
            st = A_state.pop(t)
            As = st.pop("As")
            x_t = st.pop("x_t")
            vcur = vslot[t % NSLOT]
            vprv = vslot[(t - 1) % NSLOT]
            PTsb = work.tile([128, 4, 128], F32R, name="PTsb")
            for p in range(4):
                A = As[p]
                pp = psP.tile([128, 512], F32, name="ppP", tag="ppP")
                for half in range(2):
                    hd = 2 * p + half
                    base = 256 * half
                    lo, hi = 64 * half, 64 * half + 64
                    AT = atp.tile([128, 2 * WIN], F16, name="AT")
                    if t > 0:
                        ptp = psT.tile([128, 512], F32, name="ptA", tag="ptA")
                        ptpf = ptp[:, 0:64].bitcast(F16)
                        nc.tensor.transpose(ptpf, A[:, base:base + 128], eye_f16)
                        nc.scalar.copy(out=AT[:, 0:128], in_=ptpf)
                    ptc = psT.tile([128, 512], F32, name="ptA", tag="ptA")
                    ptcf = ptc[:, 0:64].bitcast(F16)
                    nc.tensor.transpose(ptcf, A[:, base + 128:base + 256], eye_f16)
                    nc.scalar.copy(out=AT[:, 128:256], in_=ptcf)
                    vsl = slice(hd * DHEAD, (hd + 1) * DHEAD)
                    if t > 0:
                        nc.tensor.matmul(pp[lo:hi, 0:128], vprv[:, vsl],
                                         AT[:, 0:128], start=True, stop=False)
                        nc.tensor.matmul(pp[lo:hi, 0:128], vcur[:, vsl],
                                         AT[:, 128:256], start=False, stop=True)
                    else:
                        nc.tensor.matmul(pp[lo:hi, 0:128], vcur[:, vsl],
                                         AT[:, 128:256], start=True, stop=True)
                nc.scalar.copy(out=PTsb[:, p, :], in_=pp[:, 0:128])

            py = psP.tile([128, 512], F32, name="ppP", tag="ppP")
            for p in range(4):
                nc.tensor.matmul(py, PTsb[:, p, :], wo_sb[:, p, :],
                                 start=(p == 0), stop=(p == 3))
            if has_out_bias:
                nc.vector.tensor_add(py, py, bias_sb[:, 3 * DIM:4 * DIM])
            nc.vector.tensor_add(x2s[t], x_t, py)

        B_state = {}

        def sb1(t):
            """LN2 + h2T."""
            x2 = x2s[t]
            mv2, rstd2 = layernorm_stats(x2, "ln2")
            h2 = work.tile([128, DIM], BF16, name="h2")
            nc.vector.tensor_scalar(out=h2, in0=x2, scalar1=mv2[:, 0:1],
                                    scalar2=rstd2, op0=ALU.subtract, op1=ALU.mult)
            h2T = work.tile([128, 4, 128], BF16, name="h2T")
            for k in range(4):
                pt = psT.tile([128, 512], F32, name="ptA", tag="ptA")
                ptb = pt[:, 0:64].bitcast(BF16)
                nc.tensor.transpose(ptb, h2[:, k * 128:(k + 1) * 128], eye_bf)
                nc.scalar.copy(out=h2T[:, k, :], in_=ptb)
            B_state[t] = h2T

        def sb2(t):
            """FF1 (transposed) + gelu + FF2 + residual + store."""
            x2 = x2s[t]
            h2T = B_state.pop(t)
            g_sb = work.tile([128, 16, 128], BF16, name="g_sb")
            py2 = psP.tile([128, 512], F32, name="ppP", tag="ppP")
            for c in range(16):
                pg = psT.tile([128, 512], F32, name="ptA", tag="ptA")
                for k in range(4):
                    nc.tensor.matmul(pg[:, 0:128],
                                     wf1_sb[:, k, c * 128:(c + 1) * 128],
                                     h2T[:, k, :], start=(k == 0), stop=(k == 3))
                if has_ff_bias:
                    nc.scalar.activation(out=g_sb[:, c, :], in_=pg[:, 0:128],
                                         func=AF.Gelu, bias=bias_ffT[:, c:c + 1])
                else:
                    nc.scalar.activation(out=g_sb[:, c, :], in_=pg[:, 0:128],
                                         func=AF.Gelu)
                nc.tensor.matmul(py2, g_sb[:, c, :], wf2_sb[:, c, :],
                                 start=(c == 0), stop=(c == 15))
            out_t = work.tile([128, DIM], F32, name="out_t")
            nc.vector.tensor_add(out_t, x2, py2)
            nc.sync.dma_start(out=out_d[t * 128:(t + 1) * 128, :], in_=out_t)

        # software-pipelined emission
        for r in range(NT + 3):
            for fn, off in ((sa1, 0), (sa2, 1), (sa3, 2), (sa4, 3)):
                i = r - off
                if 0 <= i < NT:
                    fn(i)
        for r in range(NT + 1):
            if r < NT:
                sb1(r)
            if r >= 1:
                sb2(r - 1)

    nc.compile()
    return nc


_CACHE = {}


def prepare(x, w_qkv, q_scale, k_scale, w_out, b_out, ln1_g, ln1_b,
            ff_ln_g, ff_ln_b, w_ff1, w_ff2):
    x = np.asarray(x, np.float32)

    _bf = mybir.dt.np(BF16)
    _f16 = np.float16

    # ---- host-side folding ----
    ln1_g = np.asarray(ln1_g, np.float32)
    ln1_b = np.asarray(ln1_b, np.float32)
    ff_ln_g = np.asarray(ff_ln_g, np.float32)
    ff_ln_b = np.asarray(ff_ln_b, np.float32)
    w_qkv = np.asarray(w_qkv, np.float32)
    w_ff1 = np.asarray(w_ff1, np.float32)
    wqkvT = np.ascontiguousarray((w_qkv * ln1_g[None, :]).T).astype(_bf)
    woutT = np.ascontiguousarray(np.asarray(w_out, np.float32).T)     # (512,512)
    wff1T = np.ascontiguousarray((w_ff1 * ff_ln_g[None, :]).T).astype(_bf)
    wff2T = np.ascontiguousarray(np.asarray(w_ff2, np.float32).T).astype(_bf)
    bias_qkv = w_qkv @ ln1_b                                          # (1536,)
    bias_ff = w_ff1 @ ff_ln_b                                         # (2048,)
    b_out = np.asarray(b_out, np.float32)
    has_qkv_bias = bool(np.any(bias_qkv))
    has_ff_bias = bool(np.any(bias_ff))
    has_out_bias = bool(np.any(b_out))
    biases = np.concatenate([bias_qkv, b_out, bias_ff]).astype(np.float32)

    # rope tables with q/k scales baked in
    pos = np.arange(NTOK, dtype=np.float32)
    inv_freq = 1.0 / (10000.0 ** (np.arange(0, DHEAD, 2, dtype=np.float32) / DHEAD))
    freqs = pos[:, None] * inv_freq
    emb = np.concatenate([freqs, freqs], axis=-1)                     # (NTOK, 64)
    cos, sin = np.cos(emb), np.sin(emb)
    qs = np.asarray(q_scale, np.float32)
    ks = np.asarray(k_scale, np.float32)
    rp = np.concatenate([qs[32:], qs[:32]])                           # rotperm
    kp = np.concatenate([ks[32:], ks[:32]])
    sgn = np.concatenate([-np.ones(32, np.float32), np.ones(32, np.float32)])
    # No QK_SCALE here: the device folds it into rn_q (see build_program).
    qcos = cos * qs[None, :]
    qsin = sin * rp[None, :] * sgn[None, :]
    kcos = cos * ks[None, :]
    ksin = sin * kp[None, :] * sgn[None, :]
    rope = np.concatenate([qcos, qsin, kcos, ksin], axis=1).astype(np.float32)

    # additive causal mask as matmul stationary operand: psum gets
    # trineg.T = NEG * strict_upper (invalid j > i of the diagonal block)
    trineg = (np.tril(np.full((WIN, WIN), NEG, np.float32), k=-1)).astype(_f16)

    key = (has_qkv_bias, has_ff_bias, has_out_bias)
    if key not in _CACHE:
        _CACHE[key] = build_program(*key)
    nc = _CACHE[key]

    shared = dict(wqkvT=wqkvT, woutT=woutT, wff1T=wff1T, wff2T=wff2T,
                  rope=rope, trineg=trineg)
    if key != (False, False, False):
        shared["biases"] = biases
    in_maps = [dict(x=np.ascontiguousarray(x[i]), **shared) for i in range(B)]
    return nc, in_maps


def kernel(x, w_qkv, q_scale, k_scale, w_out, b_out, ln1_g, ln1_b,
           ff_ln_g, ff_ln_b, w_ff1, w_ff2, **run_kwargs):
    nc, in_maps = prepare(x, w_qkv, q_scale, k_scale, w_out, b_out, ln1_g,
                          ln1_b, ff_ln_g, ff_ln_b, w_ff1, w_ff2)
    res = run_bass_kernel_spmd(nc, in_maps, list(range(B)), **run_kwargs)
    out = np.stack([res.results[i]["out"] for i in range(B)]).astype(np.float32)
    if run_kwargs:
        return out, res
    return out
